# revision 9
# baseline (speedup 1.0000x reference)
"""GCNCheb Trainium2 kernel: out[b,n,fo] = sum_k T_k[b,n,:] @ W[k] + bias.

T_k recurrence (matrix powers P_j = L^j x with T0=P0, T1=P1, Tk=2*P_k - T_{k-2})
is linear, so the K/F_in contraction is re-expressed over pure powers with
host-precombined weights V_j:
    out = P0 (W0-W2) + P1 (W1-W3) + P2 (2 W2) + P3 (2 W3) + bias

Distribution over 8 NeuronCores: 1D row-shard of L. Core r holds the column
slice Lc_r = L[:, r*1024:(r+1)*1024] (== L[rows_r,:].T since L is symmetric),
pre-tiled on host to [4, 128, 64, 256] (quarter, partition, k-tile, col) and
held SBUF-resident in bf16 (16 MB). X is [N, B*F_in] = [8192, 128] (batch
folded into columns), pre-tiled to [128, 64, 128].

Every power step computes the TRANSPOSED shard directly:
    psumT[c, m] = sum_k X[k, c] * Lc[k, m]        (== (L @ X)^T rows of core r)
with the X k-tile as the 128-wide stationary operand and a 512-wide column
slice of Lc as the moving operand (128 matmuls/step instead of 512). psumT is
exactly the P_j^T layout the projection needs. The row-layout shard for the
next step's X is rebuilt with PE transposes and AllGathered in two half-shard
slices per step (the CC path only comes alive ~85us into the kernel, which
the ~70us L prefix hides; half-shard gathers are its ~13us sweet spot — a
full-shard gather measured 46us). X is double-buffered so gather write-backs
never wait on the previous step's readers; gather read-backs are split per
source core so the earliest k-tiles unblock consumers first. Step 2 consumes
k-tiles in gather-arrival order; step 3 skips the gather, interleaves the
column-half still waiting on the last gather, and projects each 512-column
slice as soon as it completes. The host untangles layout and adds bias.
"""

import sys

sys.path.insert(0, "/opt/trn_rl_repo")

import numpy as np

import concourse.bass as bass
import concourse.mybir as mybir
import concourse.tile as tile
from concourse import bacc, bass_utils
from concourse.masks import make_identity

B, N, F_IN, F_OUT, K = 4, 8192, 32, 64, 4
NCORES = 8
P = 128
SH = N // NCORES          # rows per core (1024)
BF = B * F_IN             # folded X columns (128)
KT = N // P               # contraction tiles (64)
MT = SH // P              # output row tiles per core (8)
QH = 2                    # output halves: (b in {2h, 2h+1}) x F_OUT = 128 partitions
SH4 = SH // 4             # quarter columns (256)

_DT = mybir.dt.bfloat16


def _np_dt():
    import ml_dtypes

    return np.dtype(ml_dtypes.bfloat16)


def build_nc():
    dt = _DT
    f32 = mybir.dt.float32

    nc = bacc.Bacc()
    # all pre-tiled on host: partition-major, fully contiguous per partition
    Lc = nc.dram_tensor("Lc", [4, P, KT, SH4], dt, kind="ExternalInput")
    X0 = nc.dram_tensor("X0", [P, KT, BF], dt, kind="ExternalInput")
    X0T = nc.dram_tensor("X0T", [BF, SH], dt, kind="ExternalInput")
    WH = nc.dram_tensor("WH", [K, QH, BF, P], dt, kind="ExternalInput")
    OUT = nc.dram_tensor("OUT", [QH, P, SH], f32, kind="ExternalOutput")

    def kts_of(mt0, nmt):
        return [r * MT + mt0 + m for r in range(NCORES) for m in range(nmt)]

    kt_A = kts_of(0, 4)       # k-tiles delivered by each step's half-A gather
    kt_B = kts_of(4, 4)
    kt_arr = kt_A + kt_B      # consumption order matching gather arrival

    with tile.TileContext(nc) as tc:
        with (
            tc.tile_pool(name="lres", bufs=1) as lres_pool,
            tc.tile_pool(name="xbuf", bufs=2) as x_pool,
            tc.tile_pool(name="ybuf", bufs=2) as y_pool,
            tc.tile_pool(name="proj", bufs=1) as proj_pool,
            tc.tile_pool(name="psum", bufs=1, space="PSUM") as psum_pool,
            tc.tile_pool(name="dram", bufs=1, space="DRAM") as dram_pool,
        ):
            # --- initial loads: X first (everything waits on it), then L in
            # quarter-pair order (q0+q1 feed half A of step 1, q2+q3 half B),
            # leading chunks kept small so the first matmuls start sooner ---
            x_cur = x_pool.tile([P, KT, BF], dt, tag="x", name="x0")
            nc.sync.dma_start(x_cur[:, :2, :], X0[:, :2, :])
            nc.sync.dma_start(x_cur[:, 2:8, :], X0[:, 2:8, :])
            nc.sync.dma_start(x_cur[:, 8:32, :], X0[:, 8:32, :])
            nc.sync.dma_start(x_cur[:, 32:, :], X0[:, 32:, :])

            lc_res = lres_pool.tile([P, 4, KT, SH4], dt, tag="lc_res")
            for hp in range(2):
                q0 = 2 * hp
                for ko, ke in ((0, 2), (2, 8), (8, 16), (16, 24), (24, 32),
                               (32, 40), (40, 48), (48, 56), (56, 64)):
                    for q in (q0, q0 + 1):
                        nc.scalar.dma_start(
                            lc_res[:, q, ko:ke, :],
                            Lc[q, :, ko:ke, :],
                        )

            whs = proj_pool.tile([P, K, QH, P], dt, tag="whs")
            nc.sync.dma_start(whs[:], WH.rearrange("k h p m -> p k h m"))
            pt0 = proj_pool.tile([P, SH], dt, tag="pt0")
            nc.sync.dma_start(pt0[:], X0T[:, :])
            ident = proj_pool.tile([P, P], dt, tag="ident")
            make_identity(nc, ident[:])

            pt = [pt0, None, None, None]

            def gather_slice(step, h, yshd, x_nxt, nmt=4):
                """DMA shard slice out, AllGather it, DMA back into x_nxt.

                h indexes nmt-sized mt blocks: nmt=4 gathers half the shard
                (steps whose output overlaps later compute), nmt=8 gathers the
                whole shard in one collective (step 1: the CC plane's wakeup +
                entry barrier ends long after the full shard is ready, so two
                half-gathers only add a serial mesh cycle).
                """
                mt0 = nmt * h
                shard = dram_pool.tile([P, nmt, BF], dt, name=f"shard{step}_{h}")
                full = dram_pool.tile(
                    [NCORES * P, nmt, BF],
                    dt,
                    addr_space="Shared",
                    name=f"full{step}_{h}",
                )
                nc.sync.dma_start(shard.opt(), yshd[:, mt0 : mt0 + nmt, :])
                nc.gpsimd.collective_compute(
                    "AllGather",
                    mybir.AluOpType.bypass,
                    replica_groups=[list(range(NCORES))],
                    ins=[shard.opt()],
                    outs=[full.opt()],
                )
                # split the readback per source core so the first k-tiles
                # unblock consumers before the full half lands
                xv = x_nxt[:].rearrange("p (r mt) f -> p r mt f", r=NCORES)
                fv = full[:].rearrange("(r p) mt f -> r p mt f", p=P)
                for r in range(NCORES):
                    nc.scalar.dma_start(
                        xv[:, r, mt0 : mt0 + nmt, :], fv[r, :, :, :]
                    )

            def half_mms(ps, h, x_src, kts, start, stop):
                """psumT[c, 512m] += X[k,c]^T-tiles contracted with Lc cols."""
                n = len(kts)
                for ki, kt in enumerate(kts):
                    nc.tensor.matmul(
                        ps[:],
                        lhsT=x_src[:, kt, :],
                        rhs=lc_res[:, 2 * h : 2 * h + 2, kt, :],
                        start=(start and ki == 0),
                        stop=(stop and ki == n - 1),
                    )

            def finish_half(step, h, ps, yshd, x_nxt):
                """pt copy, rebuild row-layout via PE transposes, gather."""
                nc.vector.tensor_copy(pt[step][:, h * 512 : (h + 1) * 512], ps[:])
                for t in range(4):
                    mt = 4 * h + t
                    tp = psum_pool.tile(
                        [P, P], dt, tag=f"tp{t % 2}", name=f"tp{step}_{mt}"
                    )
                    nc.tensor.transpose(
                        tp[:], pt[step][:, mt * P : (mt + 1) * P], ident[:]
                    )
                    nc.vector.tensor_copy(yshd[:, mt, :], tp[:])
                if x_nxt is not None:
                    gather_slice(step, h, yshd, x_nxt)

            # Projection accumulates into 4 persistent PSUM banks, one per
            # (batch-half hq, column-slice ns). Power j's contribution fires
            # as soon as pt[j] exists, so j=0,1 fill the long CC-plane wait
            # after step 1, j=2 the s2-gather wait, and only j=3 + the
            # evacuation remain on the tail.
            out_sb = proj_pool.tile([P, QH, 2, 512], f32, tag="out_sb")
            pp = {}

            def proj_mms(j, ns_list=(0, 1), stop=False):
                for ns in ns_list:
                    for hq in range(QH):
                        if (hq, ns) not in pp:
                            pp[(hq, ns)] = psum_pool.tile(
                                [P, 512], f32, tag=f"pj{hq}{ns}",
                                name=f"pp{hq}_{ns}",
                            )
                        nc.tensor.matmul(
                            pp[(hq, ns)][:],
                            lhsT=whs[:, j, hq, :],
                            rhs=pt[j][:, ns * 512 : (ns + 1) * 512],
                            start=(j == 0),
                            stop=stop,
                        )

            def proj_flush(ns):
                for hq in range(QH):
                    nc.vector.tensor_copy(out_sb[:, hq, ns, :], pp[(hq, ns)][:])
                    nc.sync.dma_start(
                        OUT[hq, :, ns * 512 : (ns + 1) * 512],
                        out_sb[:, hq, ns, :],
                    )

            # ---- step 1: X0 local, halves gated by L quarter-pair arrival.
            # One full-shard gather at the end: the CC plane (entry barrier +
            # ncfw wakeup) is dead until well after the whole shard is ready,
            # so splitting it in halves only adds a serial mesh cycle. ----
            pt[1] = proj_pool.tile([P, SH], dt, tag="pt1", name="pt1")
            yshd1 = y_pool.tile([P, MT, BF], dt, tag="yshd", name="yshd1")
            x1 = x_pool.tile([P, KT, BF], dt, tag="x", name="x1")
            ps = psum_pool.tile([P, 512], f32, tag="h0", name="ps1_0")
            half_mms(ps, 0, x_cur, list(range(KT)), True, True)
            finish_half(1, 0, ps, yshd1, None)
            psb = psum_pool.tile([P, 512], f32, tag="h1", name="ps1_1")
            half_mms(psb, 1, x_cur, list(range(KT)), True, True)
            finish_half(1, 1, psb, yshd1, None)
            gather_slice(1, 0, yshd1, x1, nmt=8)
            proj_mms(0)
            proj_mms(1)

            # ---- step 2: x1 lands in one wave (readback split per source,
            # r-major). h0's full contraction runs first so its half-gather
            # (mt0-3) overlaps h1's matmuls; h1's gather overlaps step 3. ----
            pt[2] = proj_pool.tile([P, SH], dt, tag="pt2", name="pt2")
            yshd2 = y_pool.tile([P, MT, BF], dt, tag="yshd", name="yshd2")
            x2 = x_pool.tile([P, KT, BF], dt, tag="x", name="x2")
            ps = psum_pool.tile([P, 512], f32, tag="h0", name="ps2_0")
            psb = psum_pool.tile([P, 512], f32, tag="h1", name="ps2_1")
            half_mms(ps, 0, x1, list(range(KT)), True, True)
            finish_half(2, 0, ps, yshd2, x2)
            half_mms(psb, 1, x1, list(range(KT)), True, True)
            finish_half(2, 1, psb, yshd2, x2)
            proj_mms(2)

            # ---- step 3: no gather needed; run both column-halves over the
            # already-landed half-A k-tiles first, then the B tails; only the
            # j=3 projection term + evacuation remain after each pt3 copy ----
            pt[3] = proj_pool.tile([P, SH], dt, tag="pt3", name="pt3")
            ps = psum_pool.tile([P, 512], f32, tag="h0", name="ps3_0")
            psb = psum_pool.tile([P, 512], f32, tag="h1", name="ps3_1")
            half_mms(ps, 0, x2, kt_A, True, False)
            half_mms(psb, 1, x2, kt_A, True, False)
            half_mms(ps, 0, x2, kt_B, False, True)
            nc.vector.tensor_copy(pt[3][:, 0:512], ps[:])
            half_mms(psb, 1, x2, kt_B, False, True)
            proj_mms(3, (0,), stop=True)
            proj_flush(0)
            nc.vector.tensor_copy(pt[3][:, 512:1024], psb[:])
            proj_mms(3, (1,), stop=True)
            proj_flush(1)

    nc.compile()
    return nc


_CACHED = {}


def _get_nc():
    if "nc" not in _CACHED:
        _CACHED["nc"] = build_nc()
    return _CACHED["nc"]


def _prep_inputs(x, L, weight):
    np_dt = _np_dt()
    f32 = np.float32

    X0 = np.ascontiguousarray(
        x.astype(f32).transpose(1, 0, 2).reshape(N, BF)
    )  # [N, (b,fi)]
    X0_t = np.ascontiguousarray(
        X0.reshape(KT, P, BF).transpose(1, 0, 2)
    ).astype(np_dt)  # [P, KT, BF]
    W = weight.astype(f32)
    V = np.stack(
        [W[0] - W[2], W[1] - W[3], 2.0 * W[2], 2.0 * W[3]]
    )  # [4, F_IN, F_OUT]
    # block-diagonal packing: WH[j, h, b*F_IN+fi, bl*F_OUT+fo] = V[j,fi,fo]
    # for b == 2h + bl
    WH = np.zeros((K, QH, BF, P), dtype=f32)
    for j in range(K):
        for b in range(B):
            h, bl = divmod(b, 2)
            WH[j, h, b * F_IN : (b + 1) * F_IN, bl * F_OUT : (bl + 1) * F_OUT] = V[j]
    WH = WH.astype(np_dt)

    in_maps = []
    for r in range(NCORES):
        rows = slice(r * SH, (r + 1) * SH)
        Lc_r = np.ascontiguousarray(
            L[:, rows].reshape(KT, P, 4, SH4).transpose(2, 1, 0, 3)
        ).astype(np_dt)  # [4, P, KT, SH4]
        X0T_r = np.ascontiguousarray(X0[rows, :].T).astype(np_dt)
        in_maps.append({"Lc": Lc_r, "X0": X0_t, "X0T": X0T_r, "WH": WH})
    return in_maps


def _assemble(results, bias):
    out = np.empty((B, N, F_OUT), dtype=np.float32)
    for r in range(NCORES):
        outT = results[r]["OUT"]  # [QH, 128, SH]
        for b in range(B):
            h, bl = divmod(b, 2)
            out[b, r * SH : (r + 1) * SH, :] = outT[
                h, bl * F_OUT : (bl + 1) * F_OUT, :
            ].T
    out += bias.astype(np.float32)
    return out


def run(x, L, weight, bias, trace=False):
    nc = _get_nc()
    in_maps = _prep_inputs(x, L, weight)
    last_err = None
    for attempt in range(3):
        try:
            res = bass_utils.run_bass_kernel_spmd(
                nc,
                in_maps,
                core_ids=list(range(NCORES)),
                trace=trace,
                trace_cores=list(range(NCORES)) if trace else None,
            )
            break
        except Exception as e:  # transient device wedge: reset + retry
            last_err = e
            import time

            try:
                import ctypes

                ctypes.CDLL("/opt/axon/libaxon_pjrt.so").axon_reset()
            except Exception:
                pass
            time.sleep(10)
    else:
        raise last_err
    out = _assemble(res.results, bias)
    return out, res


def kernel(x, L, weight, bias):
    out, _ = run(
        np.asarray(x), np.asarray(L), np.asarray(weight), np.asarray(bias)
    )
    return out

